# revision 25
# baseline (speedup 1.0000x reference)
"""MetaGatedTitansLayer Trainium2 kernel (v2.2, transposed-space bf16).

Data-parallel: B=256 sharded 32/core over 8 cores. The host ships each
state slab TRANSPOSED and cast to bf16, so both matvecs (mc = S@q,
pred = S@k) are plain PE streams of the resident slab with a 1-column
stationary vector -- no on-device transposes of the big state. The
rank-1 update is computed in transposed space too:
    newT = (1-alpha)*oldT + (eta*k)[j] x err[i]
via PE outer-products into PSUM and fused scalar_tensor_tensor.

v2.2: the M=1/K=1 matvec matmuls never drive the PE HAM clock-gate, so
they run at 1.2 GHz; instead of fighting that, blocks of 4 batch items
are packed into the four 32-wide column groups (mc/pred, col-tiling)
and four row groups (outer products, row-tiling) of the PE array via
tile_position, quadrupling matvec throughput. One of four update
chunks rides an identity-matmul + scaled-ACT-copy path to offload the
DVE. Weight DMAs are split into quarters for parallel DMA engines.
"""

import sys

import numpy as np

if "/opt/trn_rl_repo" not in sys.path:
    sys.path.insert(0, "/opt/trn_rl_repo")

B, D = 256, 512
NCORES = 8
LB = B // NCORES          # 32 local batch per core
GB = 16                   # MLP group size (2 groups)
NG = LB // GB
NBLK = GB // 4            # 4-item blocks inside a group
LN_EPS, L2_EPS = 1e-5, 1e-12
TD = 2 * D + 2            # 1026

_CACHE: dict = {}


def _build():
    import concourse.bass as bass
    import concourse.mybir as mybir
    import concourse.tile as tile
    from concourse import bacc
    from concourse.masks import make_identity

    f32 = mybir.dt.float32
    bf16 = mybir.dt.bfloat16
    AF = mybir.ActivationFunctionType
    OP = mybir.AluOpType
    AX = mybir.AxisListType

    nc = bacc.Bacc("TRN2", target_bir_lowering=False, debug=False,
                   num_devices=NCORES)

    # ---------------- DRAM I/O ----------------
    # oldT: per-b transposed state, laid out (b, p=128, jc=4, i=512) bf16
    oldT_d = nc.dram_tensor("oldT", [LB, 128, 4, D], bf16,
                            kind="ExternalInput").ap()
    xs_d = nc.dram_tensor("xs", [LB, D], f32, kind="ExternalInput").ap()
    it_d = nc.dram_tensor("it", [LB, D], f32, kind="ExternalInput").ap()
    wqT_d = nc.dram_tensor("wqT", [D, D], bf16, kind="ExternalInput").ap()
    w1T_d = nc.dram_tensor("w1T", [2 * D, D], bf16, kind="ExternalInput").ap()
    w2T_d = nc.dram_tensor("w2T", [D, TD], bf16, kind="ExternalInput").ap()
    wkvT_d = nc.dram_tensor("wkvT", [D, TD], bf16, kind="ExternalInput").ap()
    n1g_d = nc.dram_tensor("n1g", [D], f32, kind="ExternalInput").ap()
    n1b_d = nc.dram_tensor("n1b", [D], f32, kind="ExternalInput").ap()
    lng_d = nc.dram_tensor("lng", [D], f32, kind="ExternalInput").ap()
    lnb_d = nc.dram_tensor("lnb", [D], f32, kind="ExternalInput").ap()
    b1_d = nc.dram_tensor("b1", [D], f32, kind="ExternalInput").ap()
    b2_d = nc.dram_tensor("b2", [TD], f32, kind="ExternalInput").ap()
    bae_d = nc.dram_tensor("bae", [2], f32, kind="ExternalInput").ap()
    outT_d = nc.dram_tensor("outT", [LB, 128, 4, D], bf16,
                            kind="ExternalOutput").ap()

    def bcast(dst, src_1d):
        # DMA-replicate a 1-D DRAM vector across partitions.
        p = dst.shape[0]
        src = bass.AP(tensor=src_1d.tensor, offset=src_1d.offset,
                      ap=[[0, p]] + list(src_1d.ap))
        nc.gpsimd.dma_start(out=dst, in_=src)

    with tile.TileContext(nc) as tc, bass.ExitStack() as ctx:
        cst = ctx.enter_context(tc.tile_pool(name="cst", bufs=1))
        grp = ctx.enter_context(tc.tile_pool(name="grp", bufs=2))
        ps = ctx.enter_context(tc.tile_pool(name="ps", bufs=1, space="PSUM"))

        # ---------------- constants / weights ----------------
        # big weights split into quarters so 4 DMA engines run in parallel
        w1T = cst.tile([128, 8, D], bf16)
        w2T = cst.tile([128, 4, TD], bf16)
        wkvT = cst.tile([128, 4, TD], bf16)

        def load_big_weights():
            for c in range(4):
                nc.sync.dma_start(
                    out=w1T[:, 2 * c:2 * c + 2, :],
                    in_=w1T_d.rearrange("(c p) m -> p c m", p=128)[:, 2 * c:2 * c + 2, :])
                nc.sync.dma_start(
                    out=w2T[:, c, :],
                    in_=w2T_d.rearrange("(c p) m -> p c m", p=128)[:, c, :])
                nc.sync.dma_start(
                    out=wkvT[:, c, :],
                    in_=wkvT_d.rearrange("(c p) m -> p c m", p=128)[:, c, :])

        identf = cst.tile([128, 128], f32)
        make_identity(nc, identf)
        identb = cst.tile([128, 128], bf16)
        nc.scalar.copy(out=identb, in_=identf)
        negIb = cst.tile([GB, GB], bf16)
        nc.vector.tensor_scalar(negIb, identf[0:GB, 0:GB], -1.0, None,
                                op0=OP.mult)
        ones_r = cst.tile([1, 128], f32)
        nc.vector.memset(ones_r, 1.0)

        lngb = cst.tile([GB, D], f32); bcast(lngb, lng_d)
        lnbb = cst.tile([GB, D], f32); bcast(lnbb, lnb_d)
        b1b = cst.tile([GB, D], f32); bcast(b1b, b1_d)
        b2gb = cst.tile([GB, D], f32); bcast(b2gb, b2_d[0:D])
        b2bb = cst.tile([GB, D], f32); bcast(b2bb, b2_d[D:2 * D])
        b2aeb = cst.tile([GB, 2], f32); bcast(b2aeb, b2_d[2 * D:TD])
        baeb = cst.tile([GB, 2], f32); bcast(baeb, bae_d)

        # ---------------- helpers ----------------
        def rsqrt(y, x):
            nc.scalar.activation(out=y, in_=x, func=AF.Sqrt)
            nc.vector.reciprocal(out=y, in_=y)

        def layernorm(x, g_bc, b_bc, tag):
            p = x.shape[0]
            st = grp.tile([p, 6], f32, tag=f"st_{tag}", name=f"st_{tag}")
            mv = grp.tile([p, 2], f32, tag=f"mv_{tag}", name=f"mv_{tag}")
            rs = grp.tile([p, 1], f32, tag=f"rs_{tag}", name=f"rs_{tag}")
            nc.vector.bn_stats(out=st, in_=x)
            nc.vector.bn_aggr(out=mv, in_=st)
            nc.vector.tensor_scalar(mv[:, 1:2], mv[:, 1:2], LN_EPS, None,
                                    op0=OP.add)
            rsqrt(rs, mv[:, 1:2])
            nc.vector.tensor_scalar(x, x, mv[:, 0:1], None, op0=OP.subtract)
            nc.vector.tensor_scalar(x, x, rs, None, op0=OP.mult)
            nc.vector.tensor_tensor(out=x, in0=x, in1=g_bc[:p, :], op=OP.mult)
            nc.vector.tensor_tensor(out=x, in0=x, in1=b_bc[:p, :], op=OP.add)

        def l2row(x, tag):
            p = x.shape[0]
            sq = grp.tile([p, D], f32, tag=f"sq_{tag}", name=f"sq_{tag}")
            s = grp.tile([p, 1], f32, tag=f"s_{tag}", name=f"s_{tag}")
            ri = grp.tile([p, 1], f32, tag=f"ri_{tag}", name=f"ri_{tag}")
            nc.vector.tensor_tensor(out=sq, in0=x, in1=x, op=OP.mult)
            nc.vector.tensor_reduce(out=s, in_=sq, axis=AX.X, op=OP.add)
            rsqrt(ri, s)
            nc.vector.tensor_scalar(ri, ri, 1.0 / L2_EPS, None, op0=OP.min)
            nc.vector.tensor_scalar(x, x, ri, None, op0=OP.mult)

        def sigmoid_via_tanh(x):
            # sigmoid(x) = 0.5*(1 + tanh(x/2)); avoids a second ACT table.
            nc.vector.tensor_scalar(x, x, 0.5, None, op0=OP.mult)
            nc.scalar.activation(out=x, in_=x, func=AF.Tanh)
            nc.vector.tensor_scalar(x, x, 0.5, 0.5, op0=OP.mult, op1=OP.add)

        def t_small(dst_bT, src_rows, tag):
            """dst_bT (128, 4, P) bf16 <- transpose of src_rows (P, 512) bf16."""
            p = src_rows.shape[0]
            pT = ps.tile([128, 4, GB], bf16, tag="trp", bufs=1, name=f"pT_{tag}")
            for kc in range(4):
                nc.tensor.transpose(pT[:, kc, 0:p],
                                    src_rows[:, kc * 128:(kc + 1) * 128],
                                    identb[0:p, 0:p])
            nc.scalar.copy(out=dst_bT[:, :, 0:p], in_=pT[:, :, 0:p])

        # ---------------- phase 1: batch-level ----------------
        ph1 = tc.tile_pool(name="ph1", bufs=1)
        p1 = ph1.__enter__()
        wqT = p1.tile([128, 4, D], bf16)
        nc.sync.dma_start(out=wqT, in_=wqT_d.rearrange("(c p) m -> p c m", p=128))
        n1g32 = p1.tile([LB, D], f32); bcast(n1g32, n1g_d)
        n1b32 = p1.tile([LB, D], f32); bcast(n1b32, n1b_d)

        xsn = p1.tile([LB, D], f32)
        nc.sync.dma_start(out=xsn, in_=xs_d)
        layernorm(xsn, n1g32, n1b32, "xsn")
        xsnb = p1.tile([LB, D], bf16)
        nc.scalar.copy(out=xsnb, in_=xsn)

        # xsnT (128, 4, LB) bf16 for the MLP lhsT
        xsnT = cst.tile([128, 4, LB], bf16)
        for kc in range(4):
            pT = ps.tile([128, LB], bf16, tag="trp", bufs=1, name="pT_xsn")
            nc.tensor.transpose(pT, xsnb[:, kc * 128:(kc + 1) * 128],
                                identb[0:LB, 0:LB])
            nc.scalar.copy(out=xsnT[:, kc, :], in_=pT)

        load_big_weights()

        # q = l2norm(xsn @ w_q.T)
        q_rows = p1.tile([LB, D], f32)
        pq = ps.tile([LB, D], f32, tag="mlp", bufs=2, name="pq")
        for kc in range(4):
            nc.tensor.matmul(pq, lhsT=xsnT[:, kc, :], rhs=wqT[:, kc, :],
                             start=(kc == 0), stop=(kc == 3))
        nc.scalar.copy(out=q_rows, in_=pq)
        l2row(q_rows, "q")
        qrb = p1.tile([LB, D], bf16)
        nc.scalar.copy(out=qrb, in_=q_rows)
        qTs = []
        for kc in range(4):
            pT = ps.tile([128, LB], bf16, tag="trp", bufs=1, name="pT_q")
            nc.tensor.transpose(pT, qrb[:, kc * 128:(kc + 1) * 128],
                                identb[0:LB, 0:LB])
            qTk = cst.tile([128, LB], bf16, name=f"qT{kc}")
            nc.scalar.copy(out=qTk, in_=pT)
            qTs.append(qTk)

        # inorm after the q chain: off the group-0 critical path
        inorm = cst.tile([LB, D], f32)
        nc.sync.dma_start(out=inorm, in_=it_d)
        layernorm(inorm, n1g32, n1b32, "inorm")
        ph1.__exit__(None, None, None)

        slab = ctx.enter_context(tc.tile_pool(name="slab", bufs=24))
        outp = ctx.enter_context(tc.tile_pool(name="outp", bufs=5))

        # ---------------- groups ----------------
        for g in range(NG):
            g0 = g * GB
            slabs = []
            mcrow4s = [None] * NBLK

            # ---- stage 1: load slabs + col-tiled mc matvecs ----
            mcT_ps = ps.tile([128, 4, GB], f32, tag="trp", bufs=1,
                             name="mcT_ps")

            def mc_transpose(blk):
                mcrow4 = mcrow4s[blk]
                for m in range(4):
                    bp = 32 * m
                    for kc in range(4):
                        nc.tensor.transpose(
                            mcT_ps[:, kc, blk * 4 + m:blk * 4 + m + 1],
                            mcrow4[bp:bp + 1, kc * 128:(kc + 1) * 128],
                            identf[bp:bp + 1, bp:bp + 1],
                            tile_position=(bp, 0))

            for blk in range(NBLK):
                nats = []
                for m in range(4):
                    b = g0 + blk * 4 + m
                    nat = slab.tile([128, 4, D], bf16, tag="nat", name="nat")
                    nc.sync.dma_start(out=nat, in_=oldT_d[b])
                    slabs.append(nat)
                    nats.append(nat)
                pmc4 = ps.tile([128, D], f32, tag="rows", bufs=2, name="pmc4")
                for jc in range(4):
                    for m in range(4):
                        b = g0 + blk * 4 + m
                        nc.tensor.matmul(
                            pmc4[32 * m:32 * m + 1, :],
                            lhsT=qTs[jc][:, b:b + 1], rhs=nats[m][:, jc, :],
                            start=(jc == 0), stop=(jc == 3),
                            tile_position=(0, 32 * m))
                mcrow4 = grp.tile([128, D], f32, tag="mcrow4", name="mcrow4")
                nc.scalar.copy(out=mcrow4, in_=pmc4)
                mcrow4s[blk] = mcrow4
                if blk >= 1:
                    mc_transpose(blk - 1)
            mc_transpose(NBLK - 1)
            mcT = grp.tile([128, 4, GB], bf16, tag="mcT", name="mcT")
            nc.scalar.copy(out=mcT, in_=mcT_ps)

            # ---- stage 2: group MLP ----
            ph = ps.tile([GB, D], f32, tag="mlp", bufs=2, name="ph")
            for kc in range(8):
                lhsT = (xsnT[:, kc, g0:g0 + GB] if kc < 4
                        else mcT[:, kc - 4, :])
                nc.tensor.matmul(ph, lhsT=lhsT, rhs=w1T[:, kc, :],
                                 start=(kc == 0), stop=(kc == 7))
            hp = grp.tile([GB, D], f32, tag="hp", name="hp")
            nc.vector.tensor_tensor(out=hp, in0=ph, in1=b1b, op=OP.add)
            layernorm(hp, lngb, lnbb, "h")
            nc.vector.tensor_scalar(hp, hp, 0.0, None, op0=OP.max)  # relu
            hpb = grp.tile([GB, D], bf16, tag="hpb", name="hpb")
            nc.scalar.copy(out=hpb, in_=hp)
            hT = grp.tile([128, 4, GB], bf16, tag="hT", name="hT")
            t_small(hT, hpb, "hT")

            pg = ps.tile([GB, D], f32, tag="mlp", bufs=2, name="pg")
            pbe = ps.tile([GB, D], f32, tag="mlp", bufs=2, name="pbe")
            pae = ps.tile([GB, 2], f32, tag="rows", bufs=2, name="pae")
            for kc in range(4):
                st, sp = (kc == 0), (kc == 3)
                nc.tensor.matmul(pg, lhsT=hT[:, kc, :],
                                 rhs=w2T[:, kc, 0:D], start=st, stop=sp)
                nc.tensor.matmul(pbe, lhsT=hT[:, kc, :],
                                 rhs=w2T[:, kc, D:2 * D], start=st, stop=sp)
                nc.tensor.matmul(pae, lhsT=hT[:, kc, :],
                                 rhs=w2T[:, kc, 2 * D:TD], start=st, stop=sp)

            gate = grp.tile([GB, D], f32, tag="gate", name="gate")
            nc.vector.tensor_tensor(out=gate, in0=pg, in1=b2gb, op=OP.add)
            nc.scalar.activation(out=gate, in_=gate, func=AF.Tanh)
            nc.vector.tensor_scalar(gate, gate, 1.0, None, op0=OP.add)
            aeb = grp.tile([GB, 2], f32, tag="aeb", name="aeb")
            nc.vector.tensor_tensor(out=aeb, in0=pae, in1=b2aeb, op=OP.add)
            nc.vector.tensor_tensor(out=aeb, in0=aeb, in1=baeb, op=OP.add)
            # modulated = inorm * gate + beta  (beta = pbe + b2b)
            inog = grp.tile([GB, D], f32, tag="inog", name="inog")
            nc.gpsimd.dma_start(out=inog, in_=inorm[g0:g0 + GB, :])
            mod = grp.tile([GB, D], f32, tag="mod", name="mod")
            nc.vector.tensor_tensor(out=mod, in0=inog, in1=gate, op=OP.mult)
            nc.vector.tensor_tensor(out=mod, in0=mod, in1=pbe, op=OP.add)
            nc.vector.tensor_tensor(out=mod, in0=mod, in1=b2bb, op=OP.add)
            modb = grp.tile([GB, D], bf16, tag="modb", name="modb")
            nc.scalar.copy(out=modb, in_=mod)
            modT = grp.tile([128, 4, GB], bf16, tag="modT", name="modT")
            t_small(modT, modb, "modT")

            pk = ps.tile([GB, D], f32, tag="mlp", bufs=2, name="pk")
            pv = ps.tile([GB, D], f32, tag="mlp", bufs=2, name="pv")
            pae2 = ps.tile([GB, 2], f32, tag="rows", bufs=2, name="pae2")
            for kc in range(4):
                st, sp = (kc == 0), (kc == 3)
                nc.tensor.matmul(pk, lhsT=modT[:, kc, :],
                                 rhs=wkvT[:, kc, 0:D], start=st, stop=sp)
                nc.tensor.matmul(pv, lhsT=modT[:, kc, :],
                                 rhs=wkvT[:, kc, D:2 * D], start=st, stop=sp)
                nc.tensor.matmul(pae2, lhsT=modT[:, kc, :],
                                 rhs=wkvT[:, kc, 2 * D:TD], start=st, stop=sp)

            vvb = grp.tile([GB, D], bf16, tag="vvb", name="vvb")
            nc.scalar.copy(out=vvb, in_=pv)
            kr = grp.tile([GB, D], f32, tag="kr", name="kr")
            nc.scalar.copy(out=kr, in_=pk)
            l2row(kr, "k")
            krb = grp.tile([GB, D], bf16, tag="krb", name="krb")
            nc.scalar.copy(out=krb, in_=kr)
            kT = grp.tile([128, 4, GB], bf16, tag="kT", name="kT")
            t_small(kT, krb, "kT")

            nc.vector.tensor_tensor(out=aeb, in0=aeb, in1=pae2, op=OP.add)
            sigmoid_via_tanh(aeb)
            # oma = 1 - alpha; eoo = -eta * D^-0.5
            oma = grp.tile([GB, 1], f32, tag="oma", name="oma")
            nc.vector.tensor_scalar(oma, aeb[:, 0:1], -1.0, 1.0,
                                    op0=OP.mult, op1=OP.add)
            eoo = grp.tile([GB, 1], f32, tag="eoo", name="eoo")
            nc.vector.tensor_scalar(eoo, aeb[:, 1:2], -(float(D) ** -0.5),
                                    None, op0=OP.mult)
            # ekcat rows: [0]=eoo*k, [1]=eoo*k/oma (ACT-copy path)
            ekcat = grp.tile([GB, 2, D], bf16, tag="ekcat", name="ekcat")
            nc.vector.tensor_scalar(ekcat[:, 0, :], krb, eoo, None,
                                    op0=OP.mult)
            roma = grp.tile([GB, 1], f32, tag="roma", name="roma")
            nc.vector.reciprocal(out=roma, in_=oma)
            nc.vector.tensor_tensor(out=roma, in0=roma, in1=eoo, op=OP.mult)
            nc.vector.tensor_scalar(ekcat[:, 1, :], krb, roma, None,
                                    op0=OP.mult)
            # omab (128, GB): oma broadcast down partitions
            pomr = ps.tile([1, GB], f32, tag="rows", bufs=2, name="pomr")
            nc.tensor.transpose(pomr[0:1, :], oma, identf[0:GB, 0:GB])
            omr = grp.tile([1, GB], f32, tag="omr", name="omr")
            nc.scalar.copy(out=omr, in_=pomr)
            pomb = ps.tile([128, GB], f32, tag="trp", bufs=1, name="pomb")
            nc.tensor.matmul(pomb, lhsT=ones_r, rhs=omr, start=True, stop=True)
            omab = grp.tile([128, GB], f32, tag="omab", name="omab")
            nc.scalar.copy(out=omab, in_=pomb)

            # ---- stage 3: pred, update, store (block pipeline) ----
            e0qs = [None] * NBLK
            ekqs = [None] * NBLK

            def upd_block(blk):
                e0q, ekq2 = e0qs[blk], ekqs[blk]
                outTs = []
                for m in range(4):
                    outTs.append(outp.tile([128, 4, D], bf16, tag="outT",
                                           name="outT"))
                for jc in range(4):
                    for m in range(4):
                        bi = blk * 4 + m
                        bp = 32 * m
                        nat = slabs[bi]
                        pnew = ps.tile([128, D], f32, tag="pnew", bufs=3,
                                       name="pnew")
                        if jc < 2:
                            nc.tensor.matmul(
                                pnew,
                                lhsT=ekq2[bp:bp + 1, 0,
                                          jc * 128:(jc + 1) * 128],
                                rhs=e0q[bp:bp + 1, :], start=True, stop=True,
                                tile_position=(bp, 0))
                            nc.vector.scalar_tensor_tensor(
                                out=outTs[m][:, jc, :], in0=nat[:, jc, :],
                                scalar=omab[:, bi:bi + 1], in1=pnew,
                                op0=OP.mult, op1=OP.add)
                        else:
                            nc.tensor.matmul(
                                pnew,
                                lhsT=ekq2[bp:bp + 1, 1,
                                          jc * 128:(jc + 1) * 128],
                                rhs=e0q[bp:bp + 1, :], start=True, stop=False,
                                tile_position=(bp, 0))
                            nc.tensor.matmul(
                                pnew, lhsT=identb, rhs=nat[:, jc, :],
                                start=False, stop=True, tile_position=(0, 0),
                                skip_group_check=True)
                            nc.scalar.mul(outTs[m][:, jc, :], pnew,
                                          omab[:, bi:bi + 1])
                for m in range(4):
                    b = g0 + blk * 4 + m
                    nc.sync.dma_start(out=outT_d[b], in_=outTs[m])

            for blk in range(NBLK):
                nats = slabs[blk * 4:blk * 4 + 4]
                perr4 = ps.tile([128, D], f32, tag="rows", bufs=2,
                                name="perr4")
                for jc in range(4):
                    for m in range(4):
                        bi = blk * 4 + m
                        nc.tensor.matmul(
                            perr4[32 * m:32 * m + 1, :],
                            lhsT=kT[:, jc, bi:bi + 1], rhs=nats[m][:, jc, :],
                            start=(jc == 0), stop=False,
                            tile_position=(0, 32 * m))
                for m in range(4):
                    bi = blk * 4 + m
                    # perr = pred - v (sign folded into ekO)
                    nc.tensor.matmul(
                        perr4[32 * m:32 * m + 1, :],
                        lhsT=negIb[:, bi:bi + 1], rhs=vvb,
                        start=False, stop=True, tile_position=(0, 32 * m))
                e0q = grp.tile([128, D], bf16, tag="e0q", name="e0q")
                nc.scalar.copy(out=e0q, in_=perr4)
                e0qs[blk] = e0q
                ekq2 = grp.tile([128, 2, D], bf16, tag="ekq2", name="ekq2")
                for m in range(4):
                    bi = blk * 4 + m
                    nc.sync.dma_start(out=ekq2[32 * m:32 * m + 1, :, :],
                                       in_=ekcat[bi:bi + 1, :, :])
                ekqs[blk] = ekq2
                if blk >= 1:
                    upd_block(blk - 1)
            upd_block(NBLK - 1)
    nc.compile()
    return nc


def _prep_host(inputs):
    import ml_dtypes
    bf = ml_dtypes.bfloat16
    f = np.float32
    w_q = np.asarray(inputs["w_q"], f)
    w_k = np.asarray(inputs["w_k"], f)
    w_v = np.asarray(inputs["w_v"], f)
    w_a = np.asarray(inputs["w_alpha"], f).reshape(1, D)
    w_e = np.asarray(inputs["w_eta"], f).reshape(1, D)
    wkv = np.concatenate([w_k, w_v, w_a, w_e], axis=0)  # (1026, 512)
    com = {
        "wqT": np.ascontiguousarray(w_q.T).astype(bf),
        "w1T": np.ascontiguousarray(np.asarray(inputs["mc_w1"], f).T).astype(bf),
        "w2T": np.ascontiguousarray(np.asarray(inputs["mc_w2"], f).T).astype(bf),
        "wkvT": np.ascontiguousarray(wkv.T).astype(bf),
        "n1g": np.ascontiguousarray(np.asarray(inputs["n1_g"], f)),
        "n1b": np.ascontiguousarray(np.asarray(inputs["n1_b"], f)),
        "lng": np.ascontiguousarray(np.asarray(inputs["mc_ln_g"], f)),
        "lnb": np.ascontiguousarray(np.asarray(inputs["mc_ln_b"], f)),
        "b1": np.ascontiguousarray(np.asarray(inputs["mc_b1"], f)),
        "b2": np.ascontiguousarray(np.asarray(inputs["mc_b2"], f)),
        "bae": np.ascontiguousarray(
            np.stack([np.asarray(inputs["b_alpha"], f).reshape(()),
                      np.asarray(inputs["b_eta"], f).reshape(())])),
    }
    old = np.asarray(inputs["old_state"], f)
    # per-b transposed slab, tiled (b, p=128, jc=4, i=512), bf16
    oldT = old.transpose(0, 2, 1).astype(bf)           # (B, j, i)
    oldT = np.ascontiguousarray(
        oldT.reshape(B, 4, 128, D).transpose(0, 2, 1, 3))  # (B, 128, 4, D)
    xs = np.asarray(inputs["user_static_emb"], f)
    it = np.asarray(inputs["item_emb"], f)
    in_maps = []
    for c in range(NCORES):
        s = slice(c * LB, (c + 1) * LB)
        m = dict(com)
        m["oldT"] = np.ascontiguousarray(oldT[s])
        m["xs"] = np.ascontiguousarray(xs[s])
        m["it"] = np.ascontiguousarray(it[s])
        in_maps.append(m)
    return in_maps


def kernel(**inputs):
    from concourse import bass_utils

    if "nc" not in _CACHE:
        _CACHE["nc"] = _build()
    nc = _CACHE["nc"]
    in_maps = _prep_host(inputs)
    res = bass_utils.run_bass_kernel_spmd(nc, in_maps,
                                          core_ids=list(range(NCORES)))
    outT = np.concatenate([r["outT"] for r in res.results], axis=0)
    # (B, 128, 4, D) bf16 -> (B, D, D) f32, undoing the transposed layout
    out = outT.astype(np.float32).transpose(0, 2, 1, 3).reshape(B, D, D)
    out = out.transpose(0, 2, 1)
    return np.ascontiguousarray(out)


if __name__ == "__main__":
    pass
